# revision 6
# baseline (speedup 1.0000x reference)
"""LogSimpleSlater Trainium2 kernel.

Computes log|det(slater(rs, kpoints))| for B=4096 walkers of 128x128 trig
matrices, data-parallel over 8 NeuronCores (512 walkers/core).

Per core: walkers are processed in 4 groups of 128, one walker per SBUF
partition ("walker-major": M[w, i*128+j]).  The slater matrix is built with
broadcast tensor ops + one Sin activation, then factorized by batched
right-looking LU.  Pivoting is swap-free "window-4 bubble" partial pivoting:
row t is compare-exchanged with rows t+1..t+4 via copy_predicated, which
reaches LAPACK-fp32-level accuracy on these (very ill-conditioned) matrices.
log|det| = 0.5 * sum(ln(pivot^2)) via one fused Ln+accumulate activation.
"""

import numpy as np

B, N, DIM = 4096, 128, 3
NCORES = 8
BPC = B // NCORES          # walkers per core
NG = BPC // 128            # walker groups of 128 per core
KWIN = 4                   # bubble pivot window


def _build_bass():
    import concourse.bacc as bacc
    import concourse.mybir as mybir
    from concourse.tile import TileContext

    fp32 = mybir.dt.float32
    nc = bacc.Bacc(None, target_bir_lowering=False)

    rs_d = nc.dram_tensor("rs", [BPC, N, DIM], fp32, kind="ExternalInput")
    kpb_d = nc.dram_tensor("kpb", [128, 4 * N], fp32, kind="ExternalInput")
    out_d = nc.dram_tensor("out", [BPC], fp32, kind="ExternalOutput")

    with TileContext(nc) as tc:
        with tc.tile_pool(name="p", bufs=1) as pool:
            kpb0 = pool.tile([128, 4 * N], fp32, tag="kpb0")
            kpb = pool.tile([128, 4 * N], fp32, tag="kpb")
            nc.sync.dma_start(
                out=kpb0[:, :].rearrange("p (d j) -> p d j", j=N),
                in_=kpb_d[:, :].rearrange("p (d j) -> p d j", j=N),
            )
            # stage through DVE so build ops have a same-engine dep on kpb
            nc.vector.tensor_copy(kpb[:, :], kpb0[:, :])

            for g in range(NG):
                M = pool.tile([128, N * N], fp32, tag="M")
                tmpU = pool.tile([128, (N - 1) * (N - 1)], fp32, tag="tmpU")
                rsg = pool.tile([128, N * DIM], fp32, tag="rsg")
                tmpr = pool.tile([128, N], fp32, tag="tmpr")
                rsc = pool.tile([128, N], fp32, tag="rsc")
                sqa = pool.tile([128, 1], fp32, tag="sqa")
                mask = pool.tile([128, 1], mybir.dt.int32, tag="mask")
                hrec = pool.tile([128, 1], fp32, tag="hrec")
                pivsq = pool.tile([128, N], fp32, tag="pivsq")
                lns = pool.tile([128, N], fp32, tag="lns")
                sums = pool.tile([128, 1], fp32, tag="sums")

                nc.sync.dma_start(
                    out=rsg[:, :].rearrange("p (i d) -> p i d", d=DIM),
                    in_=rs_d[g * 128:(g + 1) * 128, :, :],
                )

                # ---- build M[w, i*128+j] = sin(kp_j . rs_i + phi_j) ----
                # rsg[w, i*3+d]; kpb[w(replicated), d*128+j] (d=3 is phi)
                M3 = M[:, :].rearrange("p (i j) -> p i j", j=N)
                IC = 32  # i-chunk
                for ic in range(0, N, IC):
                    mc = M3[:, ic:ic + IC, :]                     # [128, IC, N]
                    sh = [128, IC, N]
                    rx = rsg[:, :].rearrange("p (i d) -> p i d", d=DIM)
                    kx = kpb[:, :].rearrange("p (d j) -> p d j", j=N)
                    rxc = [rx[:, ic:ic + IC, d:d + 1].broadcast_to(sh) for d in range(3)]
                    kxc = [kx[:, d:d + 1, :].broadcast_to(sh) for d in range(4)]
                    nc.vector.tensor_mul(mc, rxc[0], kxc[0])
                    nc.vector.tensor_mul(tmpU[:, :IC * N].rearrange("p (i j) -> p i j", j=N), rxc[1], kxc[1])
                    nc.vector.tensor_add(mc, mc, tmpU[:, :IC * N].rearrange("p (i j) -> p i j", j=N))
                    nc.vector.tensor_mul(tmpU[:, :IC * N].rearrange("p (i j) -> p i j", j=N), rxc[2], kxc[2])
                    nc.vector.tensor_add(mc, mc, tmpU[:, :IC * N].rearrange("p (i j) -> p i j", j=N))
                    nc.vector.tensor_add(mc, mc, kxc[3])
                nc.scalar.activation(M[:, :], M[:, :], mybir.ActivationFunctionType.Sin)

                # ---- batched LU, window-KWIN bubble pivoting ----
                for t in range(N):
                    W = N - t
                    dg = M[:, t * N + t: t * N + t + 1]
                    for e in range(1, KWIN + 1):
                        b = t + e
                        if b >= N:
                            break
                        be = M[:, b * N + t: b * N + t + 1]
                        nc.vector.tensor_mul(sqa[:, :], dg, dg)
                        nc.vector.scalar_tensor_tensor(
                            out=mask[:, :], in0=be, scalar=be, in1=sqa[:, :],
                            op0=mybir.AluOpType.mult, op1=mybir.AluOpType.is_gt,
                        )
                        row_t = M[:, t * N + t: t * N + t + W]
                        row_b = M[:, b * N + t: b * N + t + W]
                        mw = mask[:, 0:1].broadcast_to([128, W])
                        nc.vector.tensor_copy(tmpr[:, :W], row_t)
                        nc.vector.copy_predicated(row_t, mw, row_b)
                        nc.vector.copy_predicated(row_b, mw, tmpr[:, :W])
                    nc.vector.tensor_mul(pivsq[:, t:t + 1], dg, dg)
                    if t < N - 1:
                        nc.vector.reciprocal(hrec[:, :], dg)
                        row_tr = M[:, t * N + t + 1: t * N + N]
                        nc.vector.tensor_scalar_mul(rsc[:, :W - 1], row_tr, hrec[:, 0:1])
                        colb = M3[:, t + 1:, t:t + 1].broadcast_to([128, W - 1, W - 1])
                        rowb = rsc[:, 0:W - 1].unsqueeze(1).broadcast_to([128, W - 1, W - 1])
                        tU = tmpU[:, :(W - 1) * (W - 1)].rearrange("p (i j) -> p i j", j=W - 1)
                        nc.vector.tensor_mul(tU, colb, rowb)
                        trail = M3[:, t + 1:, t + 1:]
                        nc.vector.tensor_sub(trail, trail, tU)

                # ---- logdet = 0.5 * sum ln(pivot^2) ----
                nc.scalar.activation(
                    lns[:, :], pivsq[:, :], mybir.ActivationFunctionType.Ln,
                    accum_out=sums[:, :],
                )
                nc.scalar.mul(sums[:, :], sums[:, :], 0.5)
                nc.sync.dma_start(out=out_d[g * 128:(g + 1) * 128], in_=sums[:, 0:1])

    nc.finalize()
    return nc


_NC_CACHE = None


def kernel(rs: np.ndarray, kpoints: np.ndarray) -> np.ndarray:
    global _NC_CACHE
    from concourse.bass_utils import run_bass_kernel_spmd

    rs = np.ascontiguousarray(rs, dtype=np.float32)
    kp = np.ascontiguousarray(kpoints, dtype=np.float32)

    # switches: cos for j==0 and odd j -> phi=pi/2 (cos x = sin(x+pi/2)); sin else
    phi = np.zeros(N, np.float32)
    phi[0] = np.pi / 2
    phi[1::2] = np.pi / 2
    kprow = np.concatenate([kp.T.reshape(-1), phi])        # [4*N]: kx|ky|kz|phi
    kpb = np.tile(kprow[None, :], (128, 1)).astype(np.float32)

    if _NC_CACHE is None:
        _NC_CACHE = _build_bass()
    nc = _NC_CACHE

    in_maps = [
        {"rs": rs[c * BPC:(c + 1) * BPC], "kpb": kpb}
        for c in range(NCORES)
    ]
    res = run_bass_kernel_spmd(nc, in_maps, core_ids=list(range(NCORES)))
    out = np.concatenate([res.results[c]["out"] for c in range(NCORES)])
    return out.astype(np.float32)


if __name__ == "__main__":
    rng = np.random.default_rng(0)
    rs = rng.standard_normal((B, N, DIM)).astype(np.float32)
    kp = rng.standard_normal((N, DIM)).astype(np.float32)
    print(kernel(rs, kp)[:8])


# revision 7
# speedup vs baseline: 2.0742x; 2.0742x over previous
"""LogSimpleSlater Trainium2 kernel.

Computes log|det(slater(rs, kpoints))| for B=4096 walkers of 128x128 trig
matrices, data-parallel over 8 NeuronCores (512 walkers/core).

Per core: walkers are processed in 4 groups of 128, one walker per SBUF
partition ("walker-major": M[w, i*128+j]).  The slater matrix is built with
broadcast tensor ops + one Sin activation, then factorized by batched
right-looking LU.  Pivoting is swap-free "window-4 bubble" partial pivoting:
row t is compare-exchanged with rows t+1..t+4 via copy_predicated, which
reaches LAPACK-fp32-level accuracy on these (very ill-conditioned) matrices.
log|det| = 0.5 * sum(ln(pivot^2)) via one fused Ln+accumulate activation.
"""

import numpy as np

B, N, DIM = 4096, 128, 3
NCORES = 8
BPC = B // NCORES          # walkers per core
NG = BPC // 128            # walker groups of 128 per core
KWIN = 2                   # bubble pivot window


def _build_bass():
    import concourse.bacc as bacc
    import concourse.mybir as mybir
    from concourse.tile import TileContext

    fp32 = mybir.dt.float32
    nc = bacc.Bacc(None, target_bir_lowering=False)

    rs_d = nc.dram_tensor("rs", [BPC, N, DIM], fp32, kind="ExternalInput")
    kpb_d = nc.dram_tensor("kpb", [128, 4 * N], fp32, kind="ExternalInput")
    out_d = nc.dram_tensor("out", [BPC], fp32, kind="ExternalOutput")

    with TileContext(nc) as tc:
        with tc.tile_pool(name="p", bufs=1) as pool:
            kpb0 = pool.tile([128, 4 * N], fp32, tag="kpb0")
            kpb = pool.tile([128, 4 * N], fp32, tag="kpb")
            nc.sync.dma_start(
                out=kpb0[:, :].rearrange("p (d j) -> p d j", j=N),
                in_=kpb_d[:, :].rearrange("p (d j) -> p d j", j=N),
            )
            # stage through DVE so build ops have a same-engine dep on kpb
            nc.vector.tensor_copy(kpb[:, :], kpb0[:, :])

            for g in range(NG):
                M = pool.tile([128, N * N], fp32, tag="M")
                tmpU = pool.tile([128, (N - 1) * (N - 1)], fp32, tag="tmpU")
                rsg = pool.tile([128, N * DIM], fp32, tag="rsg")
                tmpr = pool.tile([128, N], fp32, tag="tmpr")
                rsc = pool.tile([128, N], fp32, tag="rsc")
                sqa = pool.tile([128, 1], fp32, tag="sqa")
                mask = pool.tile([128, 1], mybir.dt.int32, tag="mask")
                hrec = pool.tile([128, 1], fp32, tag="hrec")
                pivsq = pool.tile([128, N], fp32, tag="pivsq")
                lns = pool.tile([128, N], fp32, tag="lns")
                sums = pool.tile([128, 1], fp32, tag="sums")

                nc.sync.dma_start(
                    out=rsg[:, :].rearrange("p (i d) -> p i d", d=DIM),
                    in_=rs_d[g * 128:(g + 1) * 128, :, :],
                )

                # ---- build M[w, i*128+j] = sin(kp_j . rs_i + phi_j) ----
                # rsg[w, i*3+d]; kpb[w(replicated), d*128+j] (d=3 is phi)
                M3 = M[:, :].rearrange("p (i j) -> p i j", j=N)
                IC = 32  # i-chunk
                for ic in range(0, N, IC):
                    mc = M3[:, ic:ic + IC, :]                     # [128, IC, N]
                    sh = [128, IC, N]
                    rx = rsg[:, :].rearrange("p (i d) -> p i d", d=DIM)
                    kx = kpb[:, :].rearrange("p (d j) -> p d j", j=N)
                    rxc = [rx[:, ic:ic + IC, d:d + 1].broadcast_to(sh) for d in range(3)]
                    kxc = [kx[:, d:d + 1, :].broadcast_to(sh) for d in range(4)]
                    nc.vector.tensor_mul(mc, rxc[0], kxc[0])
                    nc.vector.tensor_mul(tmpU[:, :IC * N].rearrange("p (i j) -> p i j", j=N), rxc[1], kxc[1])
                    nc.vector.tensor_add(mc, mc, tmpU[:, :IC * N].rearrange("p (i j) -> p i j", j=N))
                    nc.vector.tensor_mul(tmpU[:, :IC * N].rearrange("p (i j) -> p i j", j=N), rxc[2], kxc[2])
                    nc.vector.tensor_add(mc, mc, tmpU[:, :IC * N].rearrange("p (i j) -> p i j", j=N))
                    nc.vector.tensor_add(mc, mc, kxc[3])
                nc.scalar.activation(M[:, :], M[:, :], mybir.ActivationFunctionType.Sin)

                # ---- batched LU, window-KWIN bubble pivoting ----
                for t in range(N):
                    W = N - t
                    dg = M[:, t * N + t: t * N + t + 1]
                    for e in range(1, KWIN + 1):
                        b = t + e
                        if b >= N:
                            break
                        be = M[:, b * N + t: b * N + t + 1]
                        nc.vector.tensor_mul(sqa[:, :], dg, dg)
                        nc.vector.scalar_tensor_tensor(
                            out=mask[:, :], in0=be, scalar=be, in1=sqa[:, :],
                            op0=mybir.AluOpType.mult, op1=mybir.AluOpType.is_gt,
                        )
                        row_t = M[:, t * N + t: t * N + t + W]
                        row_b = M[:, b * N + t: b * N + t + W]
                        mw = mask[:, 0:1].broadcast_to([128, W])
                        nc.vector.tensor_copy(tmpr[:, :W], row_t)
                        nc.vector.copy_predicated(row_t, mw, row_b)
                        nc.vector.copy_predicated(row_b, mw, tmpr[:, :W])
                    nc.vector.tensor_mul(pivsq[:, t:t + 1], dg, dg)
                    if t < N - 1:
                        nc.vector.reciprocal(hrec[:, :], dg)
                        colb = M3[:, t + 1:, t:t + 1].broadcast_to([128, W - 1, W - 1])
                        rowb = M3[:, t:t + 1, t + 1:].broadcast_to([128, W - 1, W - 1])
                        tU = tmpU[:, :(W - 1) * (W - 1)].rearrange("p (i j) -> p i j", j=W - 1)
                        # tU = (col * (1/piv)) * row  in one fused op
                        nc.vector.scalar_tensor_tensor(
                            out=tU, in0=colb, scalar=hrec[:, 0:1], in1=rowb,
                            op0=mybir.AluOpType.mult, op1=mybir.AluOpType.mult,
                        )
                        trail = M3[:, t + 1:, t + 1:]
                        nc.vector.tensor_sub(trail, trail, tU)

                # ---- logdet = 0.5 * sum ln(pivot^2) ----
                nc.scalar.activation(
                    lns[:, :], pivsq[:, :], mybir.ActivationFunctionType.Ln,
                    accum_out=sums[:, :],
                )
                nc.scalar.mul(sums[:, :], sums[:, :], 0.5)
                nc.sync.dma_start(out=out_d[g * 128:(g + 1) * 128], in_=sums[:, 0:1])

    nc.finalize()
    return nc


_NC_CACHE = None


def kernel(rs: np.ndarray, kpoints: np.ndarray) -> np.ndarray:
    global _NC_CACHE
    from concourse.bass_utils import run_bass_kernel_spmd

    rs = np.ascontiguousarray(rs, dtype=np.float32)
    kp = np.ascontiguousarray(kpoints, dtype=np.float32)

    # switches: cos for j==0 and odd j -> phi=pi/2 (cos x = sin(x+pi/2)); sin else
    phi = np.zeros(N, np.float32)
    phi[0] = np.pi / 2
    phi[1::2] = np.pi / 2
    kprow = np.concatenate([kp.T.reshape(-1), phi])        # [4*N]: kx|ky|kz|phi
    kpb = np.tile(kprow[None, :], (128, 1)).astype(np.float32)

    if _NC_CACHE is None:
        _NC_CACHE = _build_bass()
    nc = _NC_CACHE

    in_maps = [
        {"rs": rs[c * BPC:(c + 1) * BPC], "kpb": kpb}
        for c in range(NCORES)
    ]
    res = run_bass_kernel_spmd(nc, in_maps, core_ids=list(range(NCORES)))
    out = np.concatenate([res.results[c]["out"] for c in range(NCORES)])
    return out.astype(np.float32)


if __name__ == "__main__":
    rng = np.random.default_rng(0)
    rs = rng.standard_normal((B, N, DIM)).astype(np.float32)
    kp = rng.standard_normal((N, DIM)).astype(np.float32)
    print(kernel(rs, kp)[:8])
